# revision 43
# baseline (speedup 1.0000x reference)
"""Trainium2 Bass kernel: MoE gate (group-limited greedy top-k routing).

Reference computation (per token t of 16384, fp32):
    logits = x @ W.T                       # [T, 64]
    scores = softmax(logits, -1)
    group_scores = scores.reshape(T, 8, 8).max(-1)
    keep top-3 groups, mask the rest, top-6 (values+indices) of masked scores

Sharding: data-parallel over tokens. Each of the 8 cores gets a
contiguous shard of 2048 tokens and a replicated copy of W; no
collectives. Selection decisions are made on exact fp32 logits (the
softmax is monotone per token), so only the output *weights* go through
the scalar-engine Exp table.

Per 128-token tile on each core:
  - DMA x_tile [128, 2048] (contiguous, 1 MiB) on the SP HWDGE ring
  - 16x PE transpose (via identity) -> xT chunks [128h, 128t] in PSUM
  - PSUM->SBUF copies of xT alternating between scalar/vector engines
  - 16x fp32 matmul accumulate logits [128t, 64e] in PSUM
  - routing: max8 / max_index / masked-add ops on the vector engine,
    Exp (+accumulated denominator) on the scalar engine
  - output stores ride the ACT HWDGE ring so they never head-of-line
    block the big x loads on the SP ring
"""

from contextlib import ExitStack

import numpy as np

import concourse.bacc as bacc
import concourse.bass as bass
import concourse.mybir as mybir
import concourse.tile as tile
from concourse.bass_utils import run_bass_kernel_spmd
from concourse.masks import make_identity

P = 128
HIDDEN = 2048
N_EXPERTS = 64
N_GROUP = 8
EPG = N_EXPERTS // N_GROUP
TOP_K = 6
N_CORES = 8
TOKENS_TOTAL = 16384
TOKENS_PER_CORE = TOKENS_TOTAL // N_CORES
NEG_BIG = -1.0e30

F32 = mybir.dt.float32
F32R = mybir.dt.float32r
AX = mybir.AxisListType
ALU = mybir.AluOpType
ACTF = mybir.ActivationFunctionType


def build_moe_gate_pret(
    ctx: ExitStack,
    tc,
    xt,
    w,
    idx_out,
    wts_out,
    lg_dump=None,
    group: int = 2,
    sustain: int = 0,
):
    """Variant taking x pre-laid-out as xt [n_tiles, 128, 16, 128] f32 DRAM,
    where xt[i, p, j, t] = x[i*128 + t, j*128 + p] — i.e. each 128-token
    tile stored hidden-major exactly in SBUF order (contiguous 8 KiB per
    partition per tile).

    No on-device transposes: each DMA'd tile block directly provides the
    stationary (lhsT) chunks for the 16 accumulating matmuls. The
    accumulation chains of `group` tiles are interleaved so adjacent PE
    matmuls target different PSUM banks (back-to-back accumulation into
    one bank serializes at ~2x cost).
    """
    nc = tc.nc
    n_tiles = xt.shape[0]
    n_chunks = HIDDEN // P
    assert n_tiles % group == 0

    consts = ctx.enter_context(tc.tile_pool(name="consts", bufs=1))
    xpool = ctx.enter_context(tc.tile_pool(name="xin", bufs=2 * group))
    xtp = ctx.enter_context(tc.tile_pool(name="xtp", bufs=2, space="PSUM"))
    lgp = ctx.enter_context(
        tc.tile_pool(name="lgp", bufs=min(2 * group, 6), space="PSUM")
    )
    rt = ctx.enter_context(tc.tile_pool(name="rt", bufs=3))

    x_tiles = {}
    for i in range(min(2 * group, n_tiles)):
        x_t = xpool.tile([P, n_chunks, P], F32, tag="xin")
        nc.sync.dma_start(x_t[:], xt[i])
        x_tiles[i] = x_t

    identity = consts.tile([P, P], F32)
    make_identity(nc, identity)

    # HAM primer: ~5us of dense back-to-back transposes while the first x
    # DMA streams in. Without this the PE clock stays at 1.2 GHz for the
    # whole kernel — the fp32 LDW+MM steady state alone never trips the
    # HAM activity window (measured: 107ns/inst for the full run vs 58ns
    # once warm).
    primer_sink = consts.tile([P, 1], F32)
    for b in range(48):
        pp = xtp.tile([P, P], F32, tag="xtp", name=f"prime_{b}")
        nc.tensor.transpose(pp[:], identity[:], identity[:])
        if b == 47:
            nc.vector.tensor_copy(primer_sink[:], pp[:, 0:1])

    w_sb = consts.tile([N_EXPERTS, HIDDEN], F32)
    nc.scalar.dma_start(w_sb[:], w)
    wt = consts.tile([P, n_chunks, N_EXPERTS], F32)
    for j in range(n_chunks):
        pt = xtp.tile([P, N_EXPERTS], F32, tag="xtp")
        nc.tensor.transpose(
            pt[:],
            w_sb[:, j * P : (j + 1) * P],
            identity[:N_EXPERTS, :N_EXPERTS],
        )
        nc.vector.tensor_copy(wt[:, j, :], pt[:])

    for i0 in range(0, n_tiles, group):
        xg, lgg = [], []
        for g in range(group):
            i = i0 + g
            if i in x_tiles:
                x_t = x_tiles.pop(i)
            else:
                x_t = xpool.tile([P, n_chunks, P], F32, tag="xin")
                nc.sync.dma_start(x_t[:], xt[i])
            xg.append(x_t)
            lgg.append(lgp.tile([P, N_EXPERTS], F32, tag="lgp", name=f"lg_{i}"))

        for j in range(n_chunks):
            for g in range(group):
                nc.tensor.matmul(
                    lgg[g][:],
                    xg[g][:, j, :],
                    wt[:, j, :],
                    start=(j == 0),
                    stop=(j == n_chunks - 1),
                )
            if sustain and j % sustain == sustain - 1:
                sp = xtp.tile([P, P], F32, tag="xtp", name=f"sustain_{i0}_{j}")
                nc.tensor.transpose(sp[:], identity[:], identity[:])
        for g in range(group):
            _routing_tail(tc, rt, lgg[g], idx_out, wts_out, i0 + g, lg_dump)


def build_v3(
    ctx: ExitStack,
    tc,
    xs,          # dict of x DRAM tensors (mode-dependent)
    wts_in,      # dict of wt DRAM tensors
    idx_out,     # [P, n_tiles, 6] uint32 DRAM (host un-permutes)
    wts_out,     # [P, n_tiles, 6] f32 DRAM
    mode: str = "f32",
    jg: int = 8,          # hidden-chunks per x sub-DMA
    primer: int = 32,     # HAM primer transposes
    sustain: int = 0,     # inject a transpose every `sustain` matmuls
    tpb: int = 512,
    lg_dump=None,         # optional [P, n_tiles, wide] f32 debug output
):
    """Streaming MoE gate. Host ships x pre-transposed block-major
    (x[b, p, j, t] = x_tok[b*tpb+t, j*128+p]) and wt[p, j, e] = W[e, j*128+p].
    Modes:
      f32:  xt/wt f32; 16 fp32 matmuls per block (bitwise-matches XLA PE f32)
      f32r: xt/wt f32r; 16 f32r matmuls (tf32-ish precision)
      bf16: xh/xl + wh/wl bf16; 48 bf16 matmuls (xh wh + xh wl + xl wh)
    Outputs land in SBUF staging [P, n_tiles, 8] and are stored per-block as
    contiguous [P, tiles_pb*6] slabs.
    """
    nc = tc.nc
    first = next(iter(xs.values()))
    n_blocks = first.shape[0]
    tiles_pb = tpb // P
    n_tiles = n_blocks * tiles_pb
    n_chunks = HIDDEN // P

    consts = ctx.enter_context(tc.tile_pool(name="consts", bufs=1))
    xpool = ctx.enter_context(tc.tile_pool(name="xin", bufs=1))
    lgp = ctx.enter_context(tc.tile_pool(name="lgp", bufs=3, space="PSUM"))
    ltp = ctx.enter_context(tc.tile_pool(name="ltp", bufs=2, space="PSUM"))
    xtp = ctx.enter_context(tc.tile_pool(name="xtp", bufs=1, space="PSUM"))
    rt = ctx.enter_context(tc.tile_pool(name="rt", bufs=3))

    SPLIT16 = {
        "bf16": mybir.dt.bfloat16,
        "fp16": mybir.dt.float16,
        "fp16s": mybir.dt.float16,
    }
    xdt = {"f32": F32, "f32r": F32R, **SPLIT16}[mode]

    # issue ALL x sub-DMAs up front: the whole shard fits in SBUF, the SP
    # ring drains them back-to-back at full rate, zero recycle stalls.
    # Issue order matches MM consumption order (block pairs interleaved).
    assert n_blocks % 2 == 0
    border = []
    for bp in range(0, n_blocks, 2):
        for s in range(n_chunks // jg):
            border += [(bp, s), (bp + 1, s)]
    x_sb = {}
    for name, xt in xs.items():
        parts = {}
        tile_shape = [P, jg] + list(xt.shape[3:])
        for b, s in border:
            xp = xpool.tile(tile_shape, xdt, tag=f"{name}_{b}_{s}")
            nc.sync.dma_start(xp[:], xt[b, :, s * jg : (s + 1) * jg])
            parts[(b, s)] = xp
        x_sb[name] = parts

    wt_sb = {}
    for name, w_in in wts_in.items():
        wt = consts.tile([P] + list(w_in.shape[1:]), xdt)
        nc.scalar.dma_start(wt[:], w_in)
        wt_sb[name] = wt

    identity = consts.tile([P, P], F32)
    make_identity(nc, identity)
    if mode == "fp16s":
        # transpose-identity with diag [1]*64 + [2^-11]*64: un-scales the
        # correction rows during the per-tile logit transpose
        identity2 = consts.tile([P, P], F32)
        nc.vector.tensor_copy(identity2[:N_EXPERTS, :], identity[:N_EXPERTS, :])
        nc.vector.tensor_scalar(
            identity2[N_EXPERTS:, :],
            identity[N_EXPERTS:, :],
            scalar1=float(2.0 ** -11),
            scalar2=None,
            op0=ALU.mult,
        )

    # HAM primer: dense transposes so the PE clock un-throttles early.
    primer_sink = consts.tile([P, 1], F32)
    for b in range(primer):
        pp = xtp.tile([P, P], F32, tag="xtp", name=f"prime_{b}")
        nc.tensor.transpose(pp[:], identity[:], identity[:])
        if b == primer - 1:
            nc.vector.tensor_copy(primer_sink[:], pp[:, 0:1])

    # output staging: contiguous [P, tiles_pb, 8], stored per block
    mm_count = 0

    def mm(out, lhsT, rhs, start, stop):
        nonlocal mm_count
        nc.tensor.matmul(out, lhsT, rhs, start=start, stop=stop)
        mm_count += 1
        if sustain and mm_count % sustain == 0:
            sp = xtp.tile([P, P], F32, tag="xtp", name=f"sustain_{mm_count}")
            nc.tensor.transpose(sp[:], identity[:], identity[:])

    # process blocks in pairs with the two accumulation chains interleaved
    # MM-by-MM so adjacent matmuls target different PSUM banks (back-to-back
    # accumulation into one bank serializes at ~2x cost).
    for bp in range(0, n_blocks, 2):
        pair = (bp, bp + 1)
        if mode in ("f32", "f32r"):
            # two accumulation chains interleaved MM-by-MM so adjacent
            # matmuls target different PSUM banks
            lgTs = [
                lgp.tile([N_EXPERTS, tpb], F32, tag="lgp", name=f"lgT_{b}")
                for b in pair
            ]
            parts = x_sb["x"]
            wt = wt_sb["w"]
            for j in range(n_chunks):
                for lgT, b in zip(lgTs, pair):
                    mm(
                        lgT[:],
                        wt[:, j, :],
                        parts[(b, j // jg)][:, j % jg, :],
                        j == 0,
                        j == n_chunks - 1,
                    )
        elif mode in ("bf16", "fp16"):
            # 16-bit 4-term split: stationary S_j interleaves wh/wl per expert
            # ([h0,l0,h1,l1,...], 128 cols, FWL-eligible); xh and xl both
            # stream through it, so lgT partitions hold (e,s) pairs whose sum
            # wh.T(xh+xl)+wl.T(xh+xl) = exact split product. The pair-sum
            # happens post-transpose as a size-2 tensor_reduce in the tail.
            lgTs = [
                lgp.tile([2 * N_EXPERTS, tpb], F32, tag="lgp", name=f"lgT_{b}")
                for b in pair
            ]
            x2 = x_sb["x2"]
            w2 = wt_sb["w2"]
            for j in range(n_chunks):
                for s in range(2):  # xh then xl, pair-interleaved
                    for lgT, b in zip(lgTs, pair):
                        mm(
                            lgT[:],
                            w2[:, j],
                            x2[(b, j // jg)][:, j % jg, s, :],
                            j == 0 and s == 0,
                            j == n_chunks - 1 and s == 1,
                        )
        else:
            # fp16s: scaled-low-part split. Stationary block [wh | wl*2^11];
            # xh streams the full block (rows 0:64 += wh.x_h, rows 64:128 +=
            # wl'.x_h); xl' = (x - xh)*2^11 streams wh only into rows 64:128
            # (+= wh.x_l'). Rows 64:128 thus hold 2^11*(wl.x_h + wh.x_l);
            # the 2^-11 un-scale rides the transpose identity. The dropped
            # wl.x_l term is ~2^-22 relative.
            lgTs = [
                lgp.tile([2 * N_EXPERTS, tpb], F32, tag="lgp", name=f"lgT_{b}")
                for b in pair
            ]
            x2 = x_sb["x2"]
            w2 = wt_sb["w2"]
            for j in range(n_chunks):
                for lgT, b in zip(lgTs, pair):
                    mm(
                        lgT[:],
                        w2[:, j],
                        x2[(b, j // jg)][:, j % jg, 0, :],
                        j == 0,
                        False,
                    )
                for lgT, b in zip(lgTs, pair):
                    mm(
                        lgT[N_EXPERTS:, :],
                        w2[:, j, 0],
                        x2[(b, j // jg)][:, j % jg, 1, :],
                        False,
                        j == n_chunks - 1,
                    )

        wide = 2 * N_EXPERTS if mode in ("bf16", "fp16", "fp16s") else N_EXPERTS
        tr_ident = identity2 if mode == "fp16s" else identity
        for lgT, b in zip(lgTs, pair):
            stage_i = rt.tile([P, tiles_pb, 8], mybir.dt.uint32, tag="st_i")
            stage_w = rt.tile([P, tiles_pb, 8], F32, tag="st_w")
            for g in range(tiles_pb):
                i = b * tiles_pb + g
                lt_sb = rt.tile([wide, P], F32, tag="lt_sb")
                if g % 2 == 0:
                    nc.scalar.copy(lt_sb[:], lgT[:, g * P : (g + 1) * P])
                else:
                    nc.vector.tensor_copy(lt_sb[:], lgT[:, g * P : (g + 1) * P])
                lg = ltp.tile([P, wide], F32, tag="ltp", name=f"lgt_{i}")
                nc.tensor.transpose(
                    lg[:], lt_sb[:], tr_ident[:wide, :wide]
                )
                pairing = (
                    "interleaved" if mode in ("bf16", "fp16")
                    else "block" if mode == "fp16s"
                    else None
                )
                _routing_tail_staged(
                    tc, rt, lg, stage_i, stage_w, g, paired=pairing,
                    lg_dump=None if lg_dump is None else lg_dump[:, i],
                )
            nc.scalar.dma_start(
                idx_out[:, b * tiles_pb : (b + 1) * tiles_pb, :],
                stage_i[:, :, :TOP_K],
            )
            nc.scalar.dma_start(
                wts_out[:, b * tiles_pb : (b + 1) * tiles_pb, :],
                stage_w[:, :, :TOP_K],
            )


def _routing_tail_staged(tc, rt, lg, stage_i, stage_w, g, paired=None,
                         lg_dump=None):
    """Routing tail for one 128-token tile; results land in staging tiles.
    paired: lg is [P, 128] holding (expert, split) pairs — "interleaved"
    ([e0s0, e0s1, ...]) or "block" ([s0e0..s0e63, s1e0..]); the size-2
    reduce-add forms the true logits."""
    nc = tc.nc
    L = rt.tile([P, N_EXPERTS], F32, tag="L")
    if paired == "interleaved":
        nc.vector.tensor_reduce(
            L[:],
            lg[:].rearrange("p (e s) -> p e s", s=2),
            axis=AX.X,
            op=ALU.add,
        )
    elif paired == "block":
        nc.vector.tensor_reduce(
            L[:],
            lg[:].rearrange("p (s e) -> p e s", s=2),
            axis=AX.X,
            op=ALU.add,
        )
    else:
        nc.vector.tensor_copy(L[:], lg[:])
    if lg_dump is not None:
        raw = rt.tile([P, lg.shape[1]], F32, tag="rawdump")
        nc.vector.tensor_copy(raw[:], lg[:])
        nc.scalar.dma_start(lg_dump, raw[:])

    ngmax = rt.tile([P, 1], F32, tag="ngmax")
    nc.vector.tensor_reduce(ngmax[:], L[:], axis=AX.X, op=ALU.max, negate=True)

    probs = rt.tile([P, N_EXPERTS], F32, tag="probs")
    den = rt.tile([P, 1], F32, tag="den")
    nc.scalar.activation(
        probs[:], L[:], ACTF.Exp, bias=ngmax[:], scale=1.0, accum_out=den[:]
    )

    gsc = rt.tile([P, N_GROUP], F32, tag="gsc")
    nc.vector.tensor_reduce(
        gsc[:],
        L[:].rearrange("p (g e) -> p g e", g=N_GROUP),
        axis=AX.X,
        op=ALU.max,
    )
    g8 = rt.tile([P, 8], F32, tag="g8")
    nc.vector.max(g8[:], gsc[:])
    gbias = rt.tile([P, N_GROUP], F32, tag="gbias")
    nc.vector.tensor_scalar(
        gbias[:],
        gsc[:],
        scalar1=g8[:, 2:3],
        scalar2=NEG_BIG,
        op0=ALU.is_lt,
        op1=ALU.mult,
    )
    lm = rt.tile([P, N_EXPERTS], F32, tag="lm")
    nc.vector.tensor_add(
        lm[:].rearrange("p (g e) -> p g e", g=N_GROUP),
        L[:].rearrange("p (g e) -> p g e", g=N_GROUP),
        gbias[:].to_broadcast([P, N_GROUP, EPG]),
    )

    v8 = rt.tile([P, 8], F32, tag="v8")
    nc.vector.max(v8[:], lm[:])
    nc.vector.max_index(stage_i[:, g, :], v8[:], lm[:])

    we = rt.tile([P, 8], F32, tag="we")
    nc.scalar.activation(we[:], v8[:], ACTF.Exp, bias=ngmax[:], scale=1.0)
    rden = rt.tile([P, 1], F32, tag="rden")
    nc.vector.reciprocal(rden[:], den[:])
    nc.vector.tensor_scalar_mul(stage_w[:, g, :], we[:], rden[:])


def build_moe_gate_stream2(
    ctx: ExitStack, tc, xt, wt_in, idx_out, wts_out, lg_dump=None
):
    """f32r streaming variant. Host ships:
      xt    [n_blocks, 128, 16, TPB] f32r: xt[b, p, j, t] = x[b*TPB+t, j*128+p]
      wt_in [128, 16, 64] f32r:       wt_in[p, j, e] = W[e, j*128+p]
    Per block: 16 accumulating f32r matmuls (1 cycle/row at N>=256) into a
    logitsT [64, TPB] PSUM bank; logits re-transposed per 128-token tile
    for the routing tail.
    """
    nc = tc.nc
    n_blocks = xt.shape[0]
    TPB = xt.shape[3]
    tiles_pb = TPB // P
    n_chunks = HIDDEN // P

    consts = ctx.enter_context(tc.tile_pool(name="consts", bufs=1))
    xpool = ctx.enter_context(tc.tile_pool(name="xin", bufs=8))
    lgp = ctx.enter_context(tc.tile_pool(name="lgp", bufs=3, space="PSUM"))
    ltp = ctx.enter_context(tc.tile_pool(name="ltp", bufs=2, space="PSUM"))
    rt = ctx.enter_context(tc.tile_pool(name="rt", bufs=3))

    JG = 4  # j-chunks per sub-DMA
    def load_block(b):
        parts = []
        for s in range(n_chunks // JG):
            xp = xpool.tile([P, JG, TPB], F32R, tag="xin", name=f"x_{b}_{s}")
            nc.sync.dma_start(xp[:], xt[b, :, s * JG : (s + 1) * JG, :])
            parts.append(xp)
        return parts

    x_blocks = {}
    for b in range(min(2, n_blocks)):
        x_blocks[b] = load_block(b)

    wt = consts.tile([P, n_chunks, N_EXPERTS], F32R)
    nc.scalar.dma_start(wt[:], wt_in)
    identity = consts.tile([P, P], F32)
    make_identity(nc, identity)

    for b in range(n_blocks):
        parts = x_blocks.pop(b) if b in x_blocks else load_block(b)

        lgT = lgp.tile([N_EXPERTS, TPB], F32, tag="lgp", name=f"lgT_{b}")
        for j in range(n_chunks):
            nc.tensor.matmul(
                lgT[:],
                wt[:, j, :],
                parts[j // JG][:, j % JG, :],
                start=(j == 0),
                stop=(j == n_chunks - 1),
            )

        for g in range(tiles_pb):
            i = b * tiles_pb + g
            lt_sb = rt.tile([N_EXPERTS, P], F32, tag="lt_sb")
            if g % 2 == 0:
                nc.scalar.copy(lt_sb[:], lgT[:, g * P : (g + 1) * P])
            else:
                nc.vector.tensor_copy(lt_sb[:], lgT[:, g * P : (g + 1) * P])
            lg = ltp.tile([P, N_EXPERTS], F32, tag="ltp", name=f"lgt_{i}")
            nc.tensor.transpose(
                lg[:], lt_sb[:], identity[:N_EXPERTS, :N_EXPERTS]
            )
            _routing_tail(tc, rt, lg, idx_out, wts_out, i, lg_dump)


def build_moe_gate_stream(
    ctx: ExitStack, tc, xt, w, idx_out, wts_out, lg_dump=None, mm_f32r=False
):
    """Flipped-stationarity variant: W^T chunks are the stationary operand,
    pre-transposed x streams 512 tokens per matmul (fp32 moving-operand
    max). Host lays x out as xt [n_blocks, 128, 16, 512] f32 DRAM with
    xt[b, p, j, t] = x[b*512 + t, j*128 + p].

    PE work per 512-token block: 16 accumulating matmuls into a
    logitsT [64, 512] PSUM bank; logits are then re-transposed per
    128-token tile for the routing tail. Long 1024-cycle streams keep the
    HAM activity monitor warm (short fp32 LDW/MM pairs do not register,
    leaving the PE clock at 1.2 GHz).
    """
    nc = tc.nc
    n_blocks = xt.shape[0]
    TPB = xt.shape[3]  # tokens per block (512)
    tiles_pb = TPB // P
    n_chunks = HIDDEN // P

    consts = ctx.enter_context(tc.tile_pool(name="consts", bufs=1))
    xpool = ctx.enter_context(tc.tile_pool(name="xin", bufs=8))
    xtp = ctx.enter_context(tc.tile_pool(name="xtp", bufs=2, space="PSUM"))
    lgp = ctx.enter_context(tc.tile_pool(name="lgp", bufs=3, space="PSUM"))
    ltp = ctx.enter_context(tc.tile_pool(name="ltp", bufs=2, space="PSUM"))
    rt = ctx.enter_context(tc.tile_pool(name="rt", bufs=3))

    # x arrives as 4 sub-DMAs per block (1 MiB each) so the first matmuls
    # start ~3us after kernel start
    JG = 4  # j-chunks per sub-DMA
    x_blocks = {}
    for b in range(min(2, n_blocks)):
        parts = []
        for s in range(n_chunks // JG):
            xp = xpool.tile([P, JG, TPB], F32, tag="xin", name=f"x_{b}_{s}")
            nc.sync.dma_start(
                xp[:], xt[b, :, s * JG : (s + 1) * JG, :]
            )
            parts.append(xp)
        x_blocks[b] = parts

    identity = consts.tile([P, P], F32)
    make_identity(nc, identity)

    w_sb = consts.tile([N_EXPERTS, HIDDEN], F32)
    nc.scalar.dma_start(w_sb[:], w)
    wt = consts.tile([P, n_chunks, N_EXPERTS], F32)
    for j in range(n_chunks):
        pt = xtp.tile([P, P], F32, tag="xtp", name=f"wtp_{j}")
        nc.tensor.transpose(
            pt[:, :N_EXPERTS],
            w_sb[:, j * P : (j + 1) * P],
            identity[:N_EXPERTS, :N_EXPERTS],
        )
        nc.vector.tensor_copy(wt[:, j, :], pt[:, :N_EXPERTS])

    for b in range(n_blocks):
        if b in x_blocks:
            parts = x_blocks.pop(b)
        else:
            parts = []
            for s in range(n_chunks // JG):
                xp = xpool.tile([P, JG, TPB], F32, tag="xin", name=f"x_{b}_{s}")
                nc.sync.dma_start(
                    xp[:], xt[b, :, s * JG : (s + 1) * JG, :]
                )
                parts.append(xp)

        lgT = lgp.tile([N_EXPERTS, TPB], F32, tag="lgp", name=f"lgT_{b}")
        for j in range(n_chunks):
            lhsT = wt[:, j, :]
            rhs = parts[j // JG][:, j % JG, :]
            if mm_f32r:
                lhsT = lhsT.bitcast(F32R)
                rhs = rhs.bitcast(F32R)
            nc.tensor.matmul(
                lgT[:],
                lhsT,
                rhs,
                start=(j == 0),
                stop=(j == n_chunks - 1),
            )

        for g in range(tiles_pb):
            i = b * tiles_pb + g
            lt_sb = rt.tile([N_EXPERTS, P], F32, tag="lt_sb")
            if g % 2 == 0:
                nc.scalar.copy(lt_sb[:], lgT[:, g * P : (g + 1) * P])
            else:
                nc.vector.tensor_copy(lt_sb[:], lgT[:, g * P : (g + 1) * P])
            lg = ltp.tile([P, N_EXPERTS], F32, tag="ltp", name=f"lgt_{i}")
            nc.tensor.transpose(
                lg[:], lt_sb[:], identity[:N_EXPERTS, :N_EXPERTS]
            )
            _routing_tail(tc, rt, lg, idx_out, wts_out, i, lg_dump)


def _routing_tail(tc, rt, lg, idx_out, wts_out, i, lg_dump):
    nc = tc.nc
    L = rt.tile([P, N_EXPERTS], F32, tag="L")
    nc.vector.tensor_copy(L[:], lg[:])
    if lg_dump is not None:
        nc.scalar.dma_start(lg_dump[i * P : (i + 1) * P, :], L[:])

    ngmax = rt.tile([P, 1], F32, tag="ngmax")
    nc.vector.tensor_reduce(ngmax[:], L[:], axis=AX.X, op=ALU.max, negate=True)

    probs = rt.tile([P, N_EXPERTS], F32, tag="probs")
    den = rt.tile([P, 1], F32, tag="den")
    nc.scalar.activation(
        probs[:], L[:], ACTF.Exp, bias=ngmax[:], scale=1.0, accum_out=den[:]
    )

    gsc = rt.tile([P, N_GROUP], F32, tag="gsc")
    nc.vector.tensor_reduce(
        gsc[:],
        L[:].rearrange("p (g e) -> p g e", g=N_GROUP),
        axis=AX.X,
        op=ALU.max,
    )
    g8 = rt.tile([P, 8], F32, tag="g8")
    nc.vector.max(g8[:], gsc[:])
    gbias = rt.tile([P, N_GROUP], F32, tag="gbias")
    nc.vector.tensor_scalar(
        gbias[:],
        gsc[:],
        scalar1=g8[:, 2:3],
        scalar2=NEG_BIG,
        op0=ALU.is_lt,
        op1=ALU.mult,
    )
    lm = rt.tile([P, N_EXPERTS], F32, tag="lm")
    nc.vector.tensor_add(
        lm[:].rearrange("p (g e) -> p g e", g=N_GROUP),
        L[:].rearrange("p (g e) -> p g e", g=N_GROUP),
        gbias[:].to_broadcast([P, N_GROUP, EPG]),
    )

    v8 = rt.tile([P, 8], F32, tag="v8")
    nc.vector.max(v8[:], lm[:])
    i8 = rt.tile([P, 8], mybir.dt.uint32, tag="i8")
    nc.vector.max_index(i8[:], v8[:], lm[:])

    we = rt.tile([P, 8], F32, tag="we")
    nc.scalar.activation(we[:], v8[:], ACTF.Exp, bias=ngmax[:], scale=1.0)
    rden = rt.tile([P, 1], F32, tag="rden")
    nc.vector.reciprocal(rden[:], den[:])
    wk = rt.tile([P, 8], F32, tag="wk")
    nc.vector.tensor_scalar_mul(wk[:], we[:], rden[:])

    nc.scalar.dma_start(idx_out[i * P : (i + 1) * P, :], i8[:, :TOP_K])
    nc.scalar.dma_start(wts_out[i * P : (i + 1) * P, :], wk[:, :TOP_K])


def build_moe_gate(
    ctx: ExitStack,
    tc,
    x,
    w,
    idx_out,
    wts_out,
    mm_f32r: bool = False,
    tr_f32r: bool = False,
    lg_dump=None,
):
    """Emit the per-core program.

    x:       [T, 2048] f32 DRAM (token shard)
    w:       [64, 2048] f32 DRAM (replicated router weight)
    idx_out: [T, 6] uint32 DRAM
    wts_out: [T, 6] f32 DRAM
    mm_f32r/tr_f32r: run matmuls / transposes with float32r-typed APs
    lg_dump: optional [T, 64] f32 DRAM to dump raw logits (debug)
    """
    nc = tc.nc
    T = x.shape[0]
    n_tiles = T // P
    n_chunks = HIDDEN // P

    # transposes land in [128, 512] PSUM macro-tiles (4 chunks each) so
    # PSUM->SBUF copies are coarse; matmuls then run back-to-back from a
    # per-tile staging buffer, letting the PE queue prefetch LDWEIGHTS.
    CPM = 4  # chunks per PSUM macro-tile
    n_macro = n_chunks // CPM

    consts = ctx.enter_context(tc.tile_pool(name="consts", bufs=1))
    xpool = ctx.enter_context(tc.tile_pool(name="xin", bufs=4))
    xtp = ctx.enter_context(tc.tile_pool(name="xtp", bufs=4, space="PSUM"))
    xts_pool = ctx.enter_context(tc.tile_pool(name="xts", bufs=2))
    lgp = ctx.enter_context(tc.tile_pool(name="lgp", bufs=2, space="PSUM"))
    rt = ctx.enter_context(tc.tile_pool(name="rt", bufs=3))

    # issue the first x loads before anything else so the SP ring starts
    # streaming immediately
    x_tiles = {}
    for i in range(min(4, n_tiles)):
        x_t = xpool.tile([P, HIDDEN], F32, tag="xin")
        nc.sync.dma_start(x_t[:], x[i * P : (i + 1) * P, :])
        x_tiles[i] = x_t

    identity = consts.tile([P, P], F32)
    make_identity(nc, identity)

    # --- preload W^T: wt[p, j, e] = W[e, j*128 + p] ---
    w_sb = consts.tile([N_EXPERTS, HIDDEN], F32)
    nc.scalar.dma_start(w_sb[:], w)
    wt = consts.tile([P, n_chunks, N_EXPERTS], F32)
    for j in range(n_chunks):
        pt = xtp.tile([P, CPM, P], F32, tag="xtp")
        nc.tensor.transpose(
            pt[:, 0, :N_EXPERTS],
            w_sb[:, j * P : (j + 1) * P],
            identity[:N_EXPERTS, :N_EXPERTS],
        )
        nc.vector.tensor_copy(wt[:, j, :], pt[:, 0, :N_EXPERTS])

    for i in range(n_tiles):
        if i in x_tiles:
            x_t = x_tiles.pop(i)
        else:
            x_t = xpool.tile([P, HIDDEN], F32, tag="xin")
            nc.sync.dma_start(x_t[:], x[i * P : (i + 1) * P, :])

        xts = xts_pool.tile([P, n_chunks, P], F32, tag="xts")
        for m in range(n_macro):
            ptile = xtp.tile([P, CPM, P], F32, tag="xtp")
            for c in range(CPM):
                j = m * CPM + c
                nc.tensor.transpose(
                    ptile[:, c, :], x_t[:, j * P : (j + 1) * P], identity[:]
                )
            # coarse PSUM->SBUF copy, alternating engines
            if m % 2 == 0:
                nc.scalar.copy(xts[:, m * CPM : (m + 1) * CPM, :], ptile[:])
            else:
                nc.vector.tensor_copy(xts[:, m * CPM : (m + 1) * CPM, :], ptile[:])

        lg = lgp.tile([P, N_EXPERTS], F32, tag="lgp")
        for j in range(n_chunks):
            nc.tensor.matmul(
                lg[:],
                xts[:, j, :],
                wt[:, j, :],
                start=(j == 0),
                stop=(j == n_chunks - 1),
            )

        # ------- routing (all selection on exact logits) -------
        L = rt.tile([P, N_EXPERTS], F32, tag="L")
        nc.vector.tensor_copy(L[:], lg[:])
        if lg_dump is not None:
            nc.scalar.dma_start(lg_dump[i * P : (i + 1) * P, :], L[:])

        ngmax = rt.tile([P, 1], F32, tag="ngmax")
        nc.vector.tensor_reduce(ngmax[:], L[:], axis=AX.X, op=ALU.max, negate=True)

        # probs is scratch; only its per-row sum (softmax denominator) is used
        probs = rt.tile([P, N_EXPERTS], F32, tag="probs")
        den = rt.tile([P, 1], F32, tag="den")
        nc.scalar.activation(
            probs[:], L[:], ACTF.Exp, bias=ngmax[:], scale=1.0, accum_out=den[:]
        )

        gsc = rt.tile([P, N_GROUP], F32, tag="gsc")
        nc.vector.tensor_reduce(
            gsc[:],
            L[:].rearrange("p (g e) -> p g e", g=N_GROUP),
            axis=AX.X,
            op=ALU.max,
        )
        g8 = rt.tile([P, 8], F32, tag="g8")
        nc.vector.max(g8[:], gsc[:])
        # additive group mask: 0 for the top-3 groups, -1e30 for the rest
        gbias = rt.tile([P, N_GROUP], F32, tag="gbias")
        nc.vector.tensor_scalar(
            gbias[:],
            gsc[:],
            scalar1=g8[:, 2:3],
            scalar2=NEG_BIG,
            op0=ALU.is_lt,
            op1=ALU.mult,
        )
        lm = rt.tile([P, N_EXPERTS], F32, tag="lm")
        nc.vector.tensor_add(
            lm[:].rearrange("p (g e) -> p g e", g=N_GROUP),
            L[:].rearrange("p (g e) -> p g e", g=N_GROUP),
            gbias[:].to_broadcast([P, N_GROUP, EPG]),
        )

        v8 = rt.tile([P, 8], F32, tag="v8")
        nc.vector.max(v8[:], lm[:])
        i8 = rt.tile([P, 8], mybir.dt.uint32, tag="i8")
        nc.vector.max_index(i8[:], v8[:], lm[:])

        # weights = exp(v - gmax) / den  for the 6 winners
        we = rt.tile([P, 8], F32, tag="we")
        nc.scalar.activation(we[:], v8[:], ACTF.Exp, bias=ngmax[:], scale=1.0)
        rden = rt.tile([P, 1], F32, tag="rden")
        nc.vector.reciprocal(rden[:], den[:])
        wk = rt.tile([P, 8], F32, tag="wk")
        nc.vector.tensor_scalar_mul(wk[:], we[:], rden[:])

        nc.scalar.dma_start(idx_out[i * P : (i + 1) * P, :], i8[:, :TOP_K])
        nc.scalar.dma_start(wts_out[i * P : (i + 1) * P, :], wk[:, :TOP_K])


def build_nc(
    tokens_per_core: int = TOKENS_PER_CORE,
    num_devices: int = N_CORES,
    mm_f32r: bool = False,
    tr_f32r: bool = False,
    dump_logits: bool = False,
    pret: bool = False,
    group: int = 2,
    sustain: int = 0,
    stream: bool = False,
    stream2: bool = False,
    v3: bool = False,
    mode: str = "f32",
    jg: int = 8,
    primer: int = 32,
    v3_sustain: int = 0,
    tpb: int = 512,
):
    nc = bacc.Bacc(
        "TRN2",
        target_bir_lowering=False,
        debug=False,
        enable_asserts=False,
        num_devices=num_devices,
    )
    n_tiles = tokens_per_core // P
    n_chunks = HIDDEN // P
    if v3:
        xdt = {
            "f32": F32,
            "f32r": F32R,
            "bf16": mybir.dt.bfloat16,
            "fp16": mybir.dt.float16,
            "fp16s": mybir.dt.float16,
        }[mode]
        xshape = [tokens_per_core // tpb, P, n_chunks, tpb]
        wshape = [P, n_chunks, N_EXPERTS]
        if mode in ("bf16", "fp16", "fp16s"):
            x2shape = [tokens_per_core // tpb, P, n_chunks, 2, tpb]
            if mode == "fp16s":
                w2shape = [P, n_chunks, 2, N_EXPERTS]
            else:
                w2shape = [P, n_chunks, N_EXPERTS, 2]
            xs_t = {"x2": nc.dram_tensor("x2", x2shape, xdt, kind="ExternalInput")}
            wts_t = {"w2": nc.dram_tensor("w2", w2shape, xdt, kind="ExternalInput")}
        else:
            xs_t = {"x": nc.dram_tensor("x", xshape, xdt, kind="ExternalInput")}
            wts_t = {"w": nc.dram_tensor("wt", wshape, xdt, kind="ExternalInput")}
        idx = nc.dram_tensor(
            "idx", [P, n_tiles, TOP_K], mybir.dt.uint32, kind="ExternalOutput"
        )
        wts = nc.dram_tensor("wts", [P, n_tiles, TOP_K], F32, kind="ExternalOutput")
        lgd = None
        if dump_logits:
            wide_n = 2 * N_EXPERTS if mode in ("bf16", "fp16", "fp16s") else N_EXPERTS
            lgd = nc.dram_tensor(
                "lg", [P, n_tiles, wide_n], F32, kind="ExternalOutput"
            ).ap()
        with tile.TileContext(nc) as tc, ExitStack() as ctx:
            build_v3(
                ctx, tc,
                {k: v.ap() for k, v in xs_t.items()},
                {k: v.ap() for k, v in wts_t.items()},
                idx.ap(), wts.ap(),
                mode=mode, jg=jg, primer=primer, sustain=v3_sustain, tpb=tpb,
                lg_dump=lgd,
            )
        nc.compile()
        return nc
    if stream2:
        x = nc.dram_tensor(
            "x", [tokens_per_core // tpb, P, n_chunks, tpb], F32R,
            kind="ExternalInput",
        )
        wt_in = nc.dram_tensor(
            "wt", [P, n_chunks, N_EXPERTS], F32R, kind="ExternalInput"
        )
    elif stream:
        x = nc.dram_tensor(
            "x", [tokens_per_core // 512, P, n_chunks, 512], F32,
            kind="ExternalInput",
        )
    elif pret:
        x = nc.dram_tensor(
            "x", [n_tiles, P, n_chunks, P], F32, kind="ExternalInput"
        )
    else:
        x = nc.dram_tensor("x", [tokens_per_core, HIDDEN], F32, kind="ExternalInput")
    w = None
    if not stream2:
        w = nc.dram_tensor("w", [N_EXPERTS, HIDDEN], F32, kind="ExternalInput")
    idx = nc.dram_tensor(
        "idx", [tokens_per_core, TOP_K], mybir.dt.uint32, kind="ExternalOutput"
    )
    wts = nc.dram_tensor("wts", [tokens_per_core, TOP_K], F32, kind="ExternalOutput")
    lg_dump = None
    if dump_logits:
        lg_dump = nc.dram_tensor(
            "lg", [tokens_per_core, N_EXPERTS], F32, kind="ExternalOutput"
        ).ap()
    with tile.TileContext(nc) as tc, ExitStack() as ctx:
        if stream2:
            build_moe_gate_stream2(
                ctx, tc, x.ap(), wt_in.ap(), idx.ap(), wts.ap(), lg_dump=lg_dump
            )
        elif stream:
            build_moe_gate_stream(
                ctx, tc, x.ap(), w.ap(), idx.ap(), wts.ap(), lg_dump=lg_dump,
                mm_f32r=mm_f32r,
            )
        elif pret:
            build_moe_gate_pret(
                ctx, tc, x.ap(), w.ap(), idx.ap(), wts.ap(), lg_dump=lg_dump,
                group=group, sustain=sustain,
            )
        else:
            build_moe_gate(
                ctx,
                tc,
                x.ap(),
                w.ap(),
                idx.ap(),
                wts.ap(),
                mm_f32r=mm_f32r,
                tr_f32r=tr_f32r,
                lg_dump=lg_dump,
            )
    nc.compile()
    return nc


_NC_CACHE = None


def _get_nc():
    global _NC_CACHE
    if _NC_CACHE is None:
        _NC_CACHE = build_nc(pret=True)
    return _NC_CACHE


def shard_pret(xs: np.ndarray) -> list[np.ndarray]:
    """Token-shard xs [16384, 2048] and lay each shard out SBUF-ordered:
    out[c][i, p, j, t] = xs[c*2048 + i*128 + t, j*128 + p]."""
    n_tiles = TOKENS_PER_CORE // P
    v = xs.reshape(N_CORES, n_tiles, P, HIDDEN // P, P)  # [c, i, t, j, p]
    v = v.transpose(0, 1, 4, 3, 2)  # [c, i, p, j, t]
    return [np.ascontiguousarray(v[c]) for c in range(N_CORES)]


def shard_stream(xs: np.ndarray) -> list[np.ndarray]:
    """Token-shard and lay out block-major for the streaming variant:
    out[c][b, p, j, t] = xs[c*2048 + b*512 + t, j*128 + p]."""
    v = xs.reshape(N_CORES, TOKENS_PER_CORE // 512, 512, HIDDEN // P, P)
    v = v.transpose(0, 1, 4, 3, 2)  # [c, b, p, j, t]
    return [np.ascontiguousarray(v[c]) for c in range(N_CORES)]


def shard_stream2(xs: np.ndarray, tpb: int = 512) -> list[np.ndarray]:
    """Token-shard and lay out block-major: out[c][b, p, j, t] =
    xs[c*TPC + b*tpb + t, j*128 + p]."""
    v = xs.reshape(N_CORES, TOKENS_PER_CORE // tpb, tpb, HIDDEN // P, P)
    v = v.transpose(0, 1, 4, 3, 2)  # [c, b, p, j, t]
    return [np.ascontiguousarray(v[c]) for c in range(N_CORES)]


def host_wt(w: np.ndarray) -> np.ndarray:
    """wt[p, j, e] = W[e, j*128 + p]."""
    return np.ascontiguousarray(
        w.reshape(N_EXPERTS, HIDDEN // P, P).transpose(2, 1, 0)
    )


def shard_stream2_t(xs: np.ndarray, tpb: int = 512, dt=np.float32) -> list:
    v = xs.reshape(N_CORES, TOKENS_PER_CORE // tpb, tpb, HIDDEN // P, P)
    v = v.transpose(0, 1, 4, 3, 2)  # [c, b, p, j, t]
    return [np.ascontiguousarray(v[c].astype(dt, copy=False)) for c in range(N_CORES)]


def v3_in_maps(xs: np.ndarray, w: np.ndarray, mode: str = "f32",
               tpb: int = 512) -> list:
    """Host-side preprocessing: layout + dtype splits for the v3 builder."""
    if mode in ("bf16", "fp16", "fp16s"):
        import ml_dtypes

        bf = ml_dtypes.bfloat16 if mode == "bf16" else np.float16
        lo_scale = 2048.0 if mode == "fp16s" else 1.0
        xh = xs.astype(bf)
        xl = ((xs - xh.astype(np.float32)) * lo_scale).astype(bf)
        wh = w.astype(bf)
        wl = ((w - wh.astype(np.float32)) * lo_scale).astype(bf)
        # x2[c][b, p, j, {h,l}, t]
        def lay(a):
            v = a.reshape(N_CORES, TOKENS_PER_CORE // tpb, tpb, HIDDEN // P, P)
            return v.transpose(0, 1, 4, 3, 2)  # [c, b, p, j, t]
        xh_l, xl_l = lay(xh), lay(xl)
        x2 = np.stack([xh_l, xl_l], axis=4)  # [c, b, p, j, 2, t]
        wh_t = host_wt(wh.astype(np.float32)).astype(bf)  # [p, j, e]
        wl_t = host_wt(wl.astype(np.float32)).astype(bf)
        ax = 2 if mode == "fp16s" else 3  # block vs interleaved expert pairs
        w2 = np.ascontiguousarray(np.stack([wh_t, wl_t], axis=ax))
        return [
            {"x2": np.ascontiguousarray(x2[c]), "w2": w2}
            for c in range(N_CORES)
        ]
    shards = shard_stream2(xs, tpb)
    wt = host_wt(w)
    return [{"x": shards[c], "wt": wt} for c in range(N_CORES)]


def v3_gather(res) -> tuple[np.ndarray, np.ndarray]:
    """Un-permute v3 outputs [P, n_tiles, 6] -> [tokens, 6] per core, concat."""
    idxs, wtss = [], []
    for r in res.results:
        oi = r["idx"]  # [P, n_tiles, 6]
        ow = r["wts"]
        idxs.append(oi.transpose(1, 0, 2).reshape(-1, TOP_K))
        wtss.append(ow.transpose(1, 0, 2).reshape(-1, TOP_K))
    return np.concatenate(idxs, 0), np.concatenate(wtss, 0)


def run_on_cores(
    xs: np.ndarray,
    w: np.ndarray,
    trace: bool = False,
    nc=None,
    pret: bool = True,
    stream: bool = False,
    stream2: bool = False,
    v3: bool = False,
    mode: str = "f32",
    tpb: int = 512,
    **kwargs,
):
    """xs: [16384, 2048] f32; w: [64, 2048] f32. Returns BassKernelResults."""
    if nc is None:
        nc = _get_nc()
    if v3:
        in_maps = v3_in_maps(xs, w, mode=mode, tpb=tpb)
        return run_bass_kernel_spmd(
            nc, in_maps, core_ids=list(range(N_CORES)), trace=trace, **kwargs
        )
    if stream2:
        shards = shard_stream2(xs, tpb)
        wt = host_wt(w)
        in_maps = [{"x": shards[c], "wt": wt} for c in range(N_CORES)]
    else:
        if stream:
            shards = shard_stream(xs)
        elif pret:
            shards = shard_pret(xs)
        else:
            shards = [
                np.ascontiguousarray(
                    xs[c * TOKENS_PER_CORE : (c + 1) * TOKENS_PER_CORE]
                )
                for c in range(N_CORES)
            ]
        in_maps = [{"x": shards[c], "w": w} for c in range(N_CORES)]
    return run_bass_kernel_spmd(
        nc, in_maps, core_ids=list(range(N_CORES)), trace=trace, **kwargs
    )


def kernel(x: np.ndarray, weight: np.ndarray):
    xs = np.ascontiguousarray(
        np.asarray(x, dtype=np.float32).reshape(TOKENS_TOTAL, HIDDEN)
    )
    w = np.ascontiguousarray(np.asarray(weight, dtype=np.float32))
    res = run_on_cores(xs, w)
    idx = np.concatenate([r["idx"].astype(np.int32) for r in res.results], axis=0)
    wts = np.concatenate(
        [r["wts"].astype(np.float32) for r in res.results], axis=0
    )
    return idx, wts

